# revision 8
# baseline (speedup 1.0000x reference)
"""BiLSTM-CRF sequence-tagging loss on 8 Trainium2 NeuronCores — v2.

Sharding: 8 cores = 4 batch-groups x 2 LSTM directions (core 2g+d, d=1 gets
time-reversed inputs; CRF uses transposed transitions + swapped start/end).

v2 speedups over the baseline:
  - chunked LSTM recurrence: 4 parallel chunk-streams per layer (chunk
    lengths 88/56/56/56, 32-step zero-state warmup) interleaved on the
    engines, cutting the sequential chain from 256 to 88 steps per layer.
  - G (+bias) is accumulated into PSUM by identity matmuls on the idle PE,
    removing two DVE adds from the per-step critical path.
  - single sigmoid over all 12 gate chunks per step (g-gate rows of the
    weights are pre-scaled x2 on the host; tanh(g) = 2*sigma(2g)-1 is
    reconstructed on DVE), halving Activation-engine work.
  - time-reversed h copies (for the pairwise exchange) run on GpSimd.
  - chunked rank-1 CRF: 16 chunks x 8 seqs advance in one 128-column
    instruction chain (~29 steps instead of 255), with a static 2^-24
    rescale folded into exp(em) instead of data-dependent renorms.
"""

import os
import sys

import numpy as np

for _p in ("/opt/trn_rl_repo", "/root/.axon_site/_ro/trn_rl_repo"):
    if os.path.isdir(_p) and _p not in sys.path:
        sys.path.insert(0, _p)

import ml_dtypes  # noqa: E402

import concourse.bass as bass  # noqa: E402
import concourse.bacc as bacc  # noqa: E402
import concourse.tile as tile  # noqa: E402
from concourse import mybir  # noqa: E402
from concourse.bass import IndirectOffsetOnAxis  # noqa: E402
from concourse.bass_utils import run_bass_kernel_spmd  # noqa: E402
from concourse.masks import make_identity  # noqa: E402

F32 = mybir.dt.float32
BF16 = mybir.dt.bfloat16
F8 = mybir.dt.float8e4
I32 = mybir.dt.int32
AF = mybir.ActivationFunctionType
ALU = mybir.AluOpType

# problem shapes (hardcoded per contract)
B, T, V, D, C, HD = 32, 256, 30522, 768, 14, 384
L = 2
NCORES = 8
GB = 8             # sequences per core group
NT = GB * T        # tokens per core = 2048
NTILE = NT // 128  # 16 embed tiles
MCH = 12           # gate chunks of 128 (4*HD/128)
KCH = 3            # hidden chunks (HD/128)
DCH = 6            # input-dim chunks (D/128)
LN_EPS = 1e-12
PAIRS = [[0, 1], [2, 3], [4, 5], [6, 7]]

# LSTM chunking: stream 0 emits [0,46) with no warmup; streams 1..6 emit 35
# tokens each with a 12-step warmup (47 steps), so streams are balanced.
LW = 11
CHUNKS = [
    (0, 46, 0),
    (46, 81, 35),
    (81, 116, 70),
    (116, 151, 105),
    (151, 186, 140),
    (186, 221, 175),
    (221, 256, 210),
]
NSTEP = 46

# CRF chunking: 16 chunks x 16 tokens, 12-step warmup, padded token axis.
CK = 16            # crf chunks
CCL = 16           # crf chunk length
CW = 12            # crf warmup
TP = T + 2 * CW    # padded token axis: [0,12) ones-pad, [12,268) real, [268,280) ones-pad
LOG2 = float(np.log(2.0))
CRF_SCALE_BITS = 24.0  # q~_t *= 2^-24 every 6th token (t % 6 == 5)


def _bf(x):
    return np.ascontiguousarray(np.asarray(x, dtype=np.float32)).astype(ml_dtypes.bfloat16)


def _f32(x):
    return np.ascontiguousarray(np.asarray(x, dtype=np.float32))


# ---------------------------------------------------------------------------
# device program
# ---------------------------------------------------------------------------

def build_program():
    nc = bacc.Bacc("TRN2", target_bir_lowering=False, debug=False, num_devices=NCORES)

    def din(name, shape, dt):
        return nc.dram_tensor(name, shape, dt, kind="ExternalInput").ap()

    CB = C * GB  # 112: (class, seq) partition layout for the score path
    ins = dict(
        ids32=din("ids32", [NT, 1], I32),
        labf=din("labf", [1, NT], F32),          # t-major labels
        word_emb=din("word_emb", [V, D], F32),
        posty=din("posty", [T, D], F32),
        wih0T=din("wih0T", [D, 4 * HD], F8),     # g-rows x2, LN folded
        wih1oT=din("wih1oT", [HD, 4 * HD], BF16),  # layer1 own-half rows, g x2
        wih1pT=din("wih1pT", [HD, 4 * HD], F8),    # layer1 partner-half rows, g x2
        whh0T=din("whh0T", [HD, 4 * HD], BF16),  # g-rows x2
        whh1T=din("whh1T", [HD, 4 * HD], BF16),  # g-rows x2
        b01=din("b01", [128, 2 * MCH], F32),     # biases, g-part x2
        fcT=din("fcT", [D, C], BF16),            # only own-half rows used
        cpack=din("cpack", [C, 64], F32),
        offq=din("offq", [C, NT], F32),          # ln-scale offsets (+fc_b) per token
        gidx=din("gidx", [128, KCH], I32),
        gidxE=din("gidxE", [C, 1], I32),
    )

    loss_out = nc.dram_tensor("loss", [1, 2], F32, kind="ExternalOutput").ap()

    # internal DRAM for pairwise exchanges: layer-0 h (fp8) and em-partials
    ctrb0 = nc.dram_tensor("ctrb0", [KCH, 128, NT], F8)
    hall0 = nc.dram_tensor("hall0", [2, KCH, 128, NT], F8)
    ctrbE = nc.dram_tensor("ctrbE", [C, NT], F32)
    hallE = nc.dram_tensor("hallE", [2, C, NT], F32)

    with tile.TileContext(nc) as tc:
        _build_body(tc, ins, loss_out, ctrb0, hall0, ctrbE, hallE)

    nc.compile()
    return nc


def _build_body(tc, ins, loss_out, ctrb0, hall0, ctrbE, hallE):
    nc = tc.nc
    from contextlib import ExitStack

    est = ExitStack()
    pers = est.enter_context(tc.tile_pool(name="pers", bufs=1))

    def load_wih0(pool):
        wt = pool.tile([128, DCH, 4 * HD], F8, name="wih0")
        nc.sync.dma_start(out=wt[:], in_=ins["wih0T"].rearrange("(k p) m -> p k m", p=128))
        return wt

    def load_wih1(pool):
        wo = pool.tile([128, KCH, 4 * HD], BF16, name="wih1o")
        nc.sync.dma_start(out=wo[:], in_=ins["wih1oT"].rearrange("(k p) m -> p k m", p=128))
        wp = pool.tile([128, KCH, 4 * HD], F8, name="wih1p")
        nc.sync.dma_start(out=wp[:], in_=ins["wih1pT"].rearrange("(k p) m -> p k m", p=128))
        return wo, wp

    def load_whh(l, pool):
        ht = pool.tile([128, KCH, 4 * HD], BF16, name=f"whh{l}")
        src = ins["whh0T"] if l == 0 else ins["whh1T"]
        nc.sync.dma_start(out=ht[:], in_=src.rearrange("(k p) m -> p k m", p=128))
        return ht

    # scratch + absorbers: one sem wait per instruction; junctions of two
    # producers fold one producer into the consumer's clock via a tiny op.
    scr_dve = pers.tile([1, 4], F32, name="scr_dve")
    scr_gp = pers.tile([1, 4], I32, name="scr_gp")
    scr_gpf = pers.tile([1, 4], F32, name="scr_gpf")
    pabs = est.enter_context(tc.tile_pool(name="pabs", bufs=1, space="PSUM"))
    pscr = pabs.tile([1, 8], F32, name="pscr")

    def dve_touch(ap):
        nc.vector.tensor_copy(out=scr_dve[:, 0:1], in_=ap)

    def pe_touch_f32(ap_col):
        nc.tensor.matmul(out=pscr[:1, :1], lhsT=ap_col, rhs=ap_col, start=True, stop=True)

    b_sb = pers.tile([128, 2 * MCH], F32, name="b_sb")
    nc.sync.dma_start(out=b_sb[:], in_=ins["b01"])
    dve_touch(b_sb[0:1, 0:1])

    fcT_sb = pers.tile([128, DCH, C], BF16, name="fcT")
    nc.sync.dma_start(out=fcT_sb[:], in_=ins["fcT"].rearrange("(k p) m -> p k m", p=128))

    cpack_sb = pers.tile([C, 64], F32, name="cpack_sb")
    nc.sync.dma_start(out=cpack_sb[:], in_=ins["cpack"])
    dve_touch(cpack_sb[0:1, 0:1])
    E_sb = cpack_sb[:, 0:C]            # exp(trans_eff)
    ET_sb = cpack_sb[:, C:2 * C]       # exp(trans_eff)^T
    expst_sb = cpack_sb[:, 42:43]      # exp(start_eff)
    expen8_sb = cpack_sb[:, 48:56]     # exp(end_eff) replicated to 8 cols

    transT_sb = cpack_sb[:, 2 * C : 3 * C]
    stv_sb = cpack_sb[:, 43:44]
    env_sb = cpack_sb[:, 44:45]
    iota_sb = cpack_sb[:, 45:46]

    offq_sb = pers.tile([C, NT], F32, name="offq_sb")
    nc.sync.dma_start(out=offq_sb[:], in_=ins["offq"])

    gidx_sb = pers.tile([128, KCH], I32, name="gidx_sb")
    nc.sync.dma_start(out=gidx_sb[:], in_=ins["gidx"])
    nc.gpsimd.tensor_copy(out=scr_gp[:, 0:1], in_=gidx_sb[0:1, 0:1])
    gidxE_sb = pers.tile([C, 1], I32, name="gidxE_sb")
    nc.sync.dma_start(out=gidxE_sb[:], in_=ins["gidxE"])
    nc.gpsimd.tensor_copy(out=scr_gp[:, 1:2], in_=gidxE_sb[0:1, 0:1])

    ids_sb = pers.tile([128, NTILE], I32, name="ids_sb")
    nc.sync.dma_start(out=ids_sb[:], in_=ins["ids32"].rearrange("(k p) o -> p (k o)", p=128))

    ident = pers.tile([128, 128], F32, name="ident")
    make_identity(nc, ident[:])
    pe_touch_f32(ident[:, 0:1])
    identb = pers.tile([128, 128], BF16, name="identb")
    nc.vector.tensor_copy(out=identb[:], in_=ident[:])
    eps_sb = pers.tile([128, 1], F32, name="eps_sb")
    nc.vector.memset(eps_sb[:], LN_EPS)
    ones1C = pers.tile([1, C], F32, name="ones1C")
    nc.vector.memset(ones1C[:], 1.0)
    onesC1 = pers.tile([C, 1], F32, name="onesC1")
    nc.vector.memset(onesC1[:], 1.0)

    # streams are processed in groups; each group shares one PSUM bank, one
    # sigmoid, and one tanh instruction. c state ping-pongs per group.
    NSTREAM = len(CHUNKS)
    # groups control only the shared tanh; psum/sigmoid stay per-stream
    GROUPS = [list(range(i, min(i + 2, NSTREAM))) for i in range(0, NSTREAM, 2)]
    GRP_OF = {}
    for gi, g in enumerate(GROUPS):
        for j, k in enumerate(g):
            GRP_OF[k] = (gi, j)
    cgrp = [[pers.tile([128, len(g), KCH, GB], F32, name=f"cg{gi}_{p}") for p in range(2)]
            for gi, g in enumerate(GROUPS)]
    c_st = [[cgrp[GRP_OF[k][0]][p][:, GRP_OF[k][1], :, :] for p in range(2)]
            for k in range(NSTREAM)]
    h_st = [[pers.tile([128, KCH, GB], BF16, name=f"hs{k}_{p}") for p in range(2)]
            for k in range(NSTREAM)]

    # ---- embedding + LN -> xT (b-major), pipelined with the layer-0 G
    # matmuls: 4 embed tiles cover exactly the 2 sequences of one G block ----
    def s1_embed_g0(xT_sb, G4, wih):
        with tc.tile_pool(name="s1", bufs=4) as s1, tc.tile_pool(
            name="s1ps", bufs=3, space="PSUM"
        ) as s1ps, tc.tile_pool(name="g0ps", bufs=3, space="PSUM") as gps:
            posty_sb = s1.tile([128, 2, D], F32, tag="posty", name="posty_sb")
            nc.sync.dma_start(
                out=posty_sb[:], in_=ins["posty"].rearrange("(a p) d -> p a d", p=128)
            )
            dve_touch(posty_sb[0:1, 0, 0:1])
            for k in range(NTILE):
                emb = s1.tile([128, D], F32, tag="emb")
                nc.gpsimd.indirect_dma_start(
                    out=emb[:],
                    out_offset=None,
                    in_=ins["word_emb"],
                    in_offset=IndirectOffsetOnAxis(ap=ids_sb[:, k : k + 1], axis=0),
                )
                nc.vector.tensor_add(out=emb[:], in0=emb[:], in1=posty_sb[:, k % 2, :])
                stats = s1.tile([128, 3, 6], F32, tag="stats")
                embv = emb[:].rearrange("p (s q) -> p s q", s=3)
                for sg in range(3):
                    nc.vector.bn_stats(out=stats[:, sg, :], in_=embv[:, sg, :])
                mv = s1.tile([128, 2], F32, tag="mv")
                nc.vector.bn_aggr(out=mv[:], in_=stats[:])
                std = s1.tile([128, 1], F32, tag="std")
                nc.scalar.activation(out=std[:], in_=mv[:, 1:2], func=AF.Sqrt, bias=eps_sb[:])
                rstd = s1.tile([128, 1], F32, tag="rstd")
                nc.vector.reciprocal(out=rstd[:], in_=std[:])
                nmr = s1.tile([128, 1], F32, tag="nmr")
                nc.vector.tensor_scalar(
                    out=nmr[:], in0=mv[:, 0:1], scalar1=rstd[:], scalar2=-1.0,
                    op0=ALU.mult, op1=ALU.mult,
                )
                xln = s1.tile([128, D], BF16, tag="xln")
                nc.scalar.activation(
                    out=xln[:], in_=emb[:], func=AF.Identity,
                    bias=nmr[:], scale=rstd[:],
                )
                for j in range(DCH):
                    tp = s1ps.tile([128, 128], BF16, tag="tp")
                    nc.tensor.transpose(
                        out=tp[:], in_=xln[:, 128 * j : 128 * (j + 1)], identity=identb[:]
                    )
                    if j % 3 == 0:
                        nc.vector.tensor_copy(out=xT_sb[:, j, 128 * k : 128 * (k + 1)], in_=tp[:])
                    else:
                        nc.scalar.copy(out=xT_sb[:, j, 128 * k : 128 * (k + 1)], in_=tp[:])
                if k % 4 == 3:
                    # G block nb = k//4 (psum cols are (b_loc, t) b-major)
                    nb = k // 4
                    for m in range(MCH):
                        ps = gps.tile([128, 512], F32, tag="gps")
                        for jj in range(DCH // 2):
                            nc.tensor.matmul(
                                out=ps[:],
                                lhsT=wih[:, 2 * jj : 2 * jj + 2, 128 * m : 128 * (m + 1)],
                                rhs=xT_sb[:, 2 * jj : 2 * jj + 2, 512 * nb : 512 * (nb + 1)],
                                start=(jj == 0),
                                stop=(jj == DCH // 2 - 1),
                                perf_mode=mybir.MatmulPerfMode.DoubleRow,
                            )
                        out = G4[:, m, :, 2 * nb : 2 * nb + 2].rearrange("p t b -> p b t")
                        if m % 2 == 0:
                            nc.vector.tensor_scalar_add(
                                out=out, in0=ps[:].rearrange("p (b t) -> p b t", b=2),
                                scalar1=b_sb[:, 0 * MCH + m : 0 * MCH + m + 1],
                            )
                        else:
                            nc.scalar.activation(
                                out=out, in_=ps[:].rearrange("p (b t) -> p b t", b=2),
                                func=AF.Identity,
                                bias=b_sb[:, 0 * MCH + m : 0 * MCH + m + 1],
                            )

    def g_matmul_l1_own(G4, hT4, wih1o):
        # own-half contribution + bias -> G4; runs while the collective is in
        # flight. rhs blocks are t-major contiguous: 64 t x 8 b = 512 cols.
        with tc.tile_pool(name="g1ps", bufs=3, space="PSUM") as gps:
            nc.tensor.ldweights(weights=wih1o[:, 0, 0:1])
            for m in range(MCH):
                for nb in range(4):
                    ps = gps.tile([128, 512], F32, tag="gps")
                    for kk in range(KCH):
                        nc.tensor.matmul(
                            out=ps[:],
                            lhsT=wih1o[:, kk, 128 * m : 128 * (m + 1)],
                            rhs=hT4[:, kk, 64 * nb : 64 * (nb + 1), :].rearrange(
                                "p t b -> p (t b)"
                            ),
                            start=(kk == 0),
                            stop=(kk == KCH - 1),
                        )
                    if m % 2 == 0:
                        nc.vector.tensor_scalar_add(
                            out=G4[:, m, 64 * nb : 64 * (nb + 1), :],
                            in0=ps[:],
                            scalar1=b_sb[:, 1 * MCH + m : 1 * MCH + m + 1],
                        )
                    else:
                        nc.scalar.activation(
                            out=G4[:, m, 64 * nb : 64 * (nb + 1), :].rearrange(
                                "p t b -> p (t b)"
                            ),
                            in_=ps[:],
                            func=AF.Identity,
                            bias=b_sb[:, 1 * MCH + m : 1 * MCH + m + 1],
                        )

    def g_matmul_l1_partner(G4, xp4, wih1p):
        # partner-half accumulate onto G4 after the exchange lands
        with tc.tile_pool(name="g1q", bufs=3, space="PSUM") as gps:
            nc.tensor.ldweights(weights=wih1p[:, 0, 0:1])
            for m in range(MCH):
                for nb in range(4):
                    ps = gps.tile([128, 512], F32, tag="gps")
                    dst = G4[:, m, 64 * nb : 64 * (nb + 1), :].rearrange("p t b -> p (t b)")
                    if m % 2 == 1:
                        # fold the own-half (already in G4) into PSUM on the PE
                        # so the writeback is a plain copy on the Act engine
                        nc.tensor.matmul(
                            out=ps[:], lhsT=identb[:], rhs=dst,
                            start=True, stop=False, skip_group_check=True,
                        )
                    nc.tensor.matmul(
                        out=ps[:],
                        lhsT=wih1p[:, 0:2, 128 * m : 128 * (m + 1)],
                        rhs=xp4[:, 0:2, 64 * nb : 64 * (nb + 1), :].rearrange(
                            "p k t b -> p k (t b)"
                        ),
                        start=(m % 2 == 0),
                        stop=False,
                        perf_mode=mybir.MatmulPerfMode.DoubleRow,
                        skip_group_check=True,
                    )
                    nc.tensor.matmul(
                        out=ps[:],
                        lhsT=wih1p[:, 2, 128 * m : 128 * (m + 1)],
                        rhs=xp4[:, 2, 64 * nb : 64 * (nb + 1), :].rearrange(
                            "p t b -> p (t b)"
                        ),
                        start=False,
                        stop=True,
                        skip_group_check=True,
                    )
                    if m % 2 == 1:
                        nc.scalar.copy(out=dst, in_=ps[:])
                    else:
                        nc.vector.tensor_tensor(out=dst, in0=ps[:], in1=dst, op=ALU.add)

    # ---- chunked LSTM recurrence over one layer ----
    def recurrence(l, G4, hT4, hTr4, whh):
        with tc.tile_pool(name=f"r{l}", bufs=6) as rp, tc.tile_pool(
            name=f"r{l}ps", bufs=1, space="PSUM"
        ) as rps:
            nc.tensor.ldweights(weights=whh[:, 0, 0:1])
            for step in range(NSTEP):
                par = step & 1
                first = step == 0  # every stream starts (warmup or t=0) at step 0

                def tb(k):
                    t0, t1, ws = CHUNKS[k]
                    start_t = t0 if k == 0 else ws
                    t = start_t + step
                    return t, t >= t0

                def chain_a(k):
                    t, emit = tb(k)
                    c_prev, c_new = c_st[k][1 - par], c_st[k][par]
                    ps = rps.tile([128, MCH, GB], F32, tag=f"ps{k}")
                    for m in range(MCH):
                        nc.tensor.matmul(
                            out=ps[:, m, :],
                            lhsT=identb[:],
                            rhs=G4[:, m, t, :],
                            start=(m == 0),
                            stop=(first and m == MCH - 1),
                            skip_group_check=True,
                        )
                    if not first:
                        rd_hT = emit and t - 1 >= CHUNKS[k][0]
                        for kk in range(KCH):
                            rh = (
                                hT4[:, kk, t - 1, :]
                                if rd_hT
                                else h_st[k][1 - par][:, kk, :]
                            )
                            for m in range(MCH):
                                nc.tensor.matmul(
                                    out=ps[:, m, :],
                                    lhsT=whh[:, kk, 128 * m : 128 * (m + 1)],
                                    rhs=rh,
                                    start=False,
                                    stop=(kk == KCH - 1 and m == MCH - 1),
                                    skip_group_check=True,
                                )
                    sg = rp.tile([128, MCH, GB], F32, tag=f"sg{k}")
                    nc.scalar.activation(out=sg[:], in_=ps[:], func=AF.Sigmoid)
                    # u = tanh(g) = 2*sigma(2g) - 1 (g-rows pre-scaled x2);
                    # on GpSimd to keep DVE off the critical load
                    nc.gpsimd.tensor_scalar(
                        out=sg[:, 6:9, :], in0=sg[:, 6:9, :],
                        scalar1=2.0, scalar2=1.0, op0=ALU.mult, op1=ALU.subtract,
                    )
                    if first:
                        nc.vector.tensor_tensor(
                            out=c_new[:], in0=sg[:, 0:3, :], in1=sg[:, 6:9, :], op=ALU.mult
                        )
                    else:
                        tt1 = rp.tile([128, KCH, GB], F32, tag=f"t1{k}")
                        nc.vector.tensor_tensor(
                            out=tt1[:], in0=sg[:, 3:6, :], in1=c_prev[:], op=ALU.mult
                        )
                        tt2 = rp.tile([128, KCH, GB], F32, tag=f"t2{k}")
                        nc.vector.tensor_tensor(
                            out=tt2[:], in0=sg[:, 0:3, :], in1=sg[:, 6:9, :], op=ALU.mult
                        )
                        nc.vector.tensor_tensor(
                            out=c_new[:], in0=tt1[:], in1=tt2[:], op=ALU.add
                        )
                    return sg

                for gi, grp in enumerate(GROUPS):
                    sgl = [chain_a(k) for k in grp]
                    th = rp.tile([128, len(grp), KCH, GB], F32, tag=f"th{gi}")
                    nc.scalar.activation(out=th[:], in_=cgrp[gi][par][:], func=AF.Tanh)
                    for j, k in enumerate(grp):
                        t, emit = tb(k)
                        hdst = hT4[:, :, t, :] if emit else h_st[k][par][:]
                        nc.vector.tensor_tensor(
                            out=hdst, in0=sgl[j][:, 9:12, :], in1=th[:, j, :, :],
                            op=ALU.mult,
                        )

    def exchange(hTr4, xp4):
        nc.sync.dma_start(
            out=ctrb0.ap().rearrange("c p (t b) -> p c t b", b=GB), in_=hTr4[:]
        )
        nc.gpsimd.collective_compute(
            "AllGather",
            ALU.bypass,
            replica_groups=PAIRS,
            ins=[ctrb0.ap()],
            outs=[hall0.ap()],
        )
        rows = hall0.ap().rearrange("r c p n -> (r c p) n")
        for cch in range(KCH):
            nc.gpsimd.indirect_dma_start(
                out=xp4[:, cch, :, :].rearrange("p t b -> p (t b)"),
                out_offset=None,
                in_=rows,
                in_offset=IndirectOffsetOnAxis(ap=gidx_sb[:, cch : cch + 1], axis=0),
            )

    # ---- layer pipeline ----
    with tc.tile_pool(name="phh", bufs=1) as phh:
        hT0 = phh.tile([128, KCH, T, GB], BF16, name="hT0")
        hTr0 = phh.tile([128, KCH, T, GB], F8, name="hTr0")
        hT1 = phh.tile([128, KCH, T, GB], BF16, name="hT1")
        xp4 = phh.tile([128, KCH, T, GB], F8, name="xp4")
        with tc.tile_pool(name="pg", bufs=1) as pgp:
            G4 = pgp.tile([128, MCH, T, GB], BF16, name="G4")
            with tc.tile_pool(name="pr0", bufs=1) as pr0:
                whh0 = load_whh(0, pr0)
                with tc.tile_pool(name="pw0", bufs=1) as pw0:
                    wih0 = load_wih0(pw0)
                    with tc.tile_pool(name="px", bufs=1) as px:
                        xT_sb = px.tile([128, DCH, NT], F8, name="xT_sb")
                        s1_embed_g0(xT_sb, G4, wih0)
                recurrence(0, G4, hT0, hTr0, whh0)
            # time-reverse per sequence for the pairwise exchange (fp8)
            nc.vector.tensor_copy(out=hTr0[:], in_=hT0[:, :, ::-1, :])
            with tc.tile_pool(name="pw1", bufs=1) as pw1:
                wih1o, wih1p = load_wih1(pw1)
                exchange(hTr0, xp4)
                g_matmul_l1_own(G4, hT0, wih1o)
                for cch in range(KCH):
                    nc.tensor.ldweights(weights=xp4[:, cch, 0, 0:1])
                g_matmul_l1_partner(G4, xp4, wih1p)
            with tc.tile_pool(name="pr1", bufs=1) as pr1:
                whh1 = load_whh(1, pr1)
                recurrence(1, G4, hT1, None, whh1)

        # ---- emissions: own-half partial, pairwise-summed via DRAM ----
        crf_cm = tc.tile_pool(name="crf", bufs=1)
        crf = crf_cm.__enter__()
        labf_sb = crf.tile([1, NT], F32, name="labf_sb")
        nc.sync.dma_start(out=labf_sb[:], in_=ins["labf"])
        pe_touch_f32(cpack_sb[:, 0:1])
        emT = crf.tile([C, T, GB], F32, name="emT")
        xpem = crf.tile([C, T, GB], F32, name="xpem")
        with tc.tile_pool(name="emps", bufs=2, space="PSUM") as emps:
            nc.tensor.ldweights(weights=fcT_sb[:, 0, 0:1])
            for nb in range(4):
                ps = emps.tile([128, 512], F32, tag="emps")
                for kk in range(KCH):
                    nc.tensor.matmul(
                        out=ps[:C, :],
                        lhsT=fcT_sb[:, kk, :],
                        rhs=hT1[:, kk, 64 * nb : 64 * (nb + 1), :].rearrange(
                            "p t b -> p (t b)"
                        ),
                        start=(kk == 0),
                        stop=(kk == KCH - 1),
                    )
                nc.vector.tensor_copy(
                    out=emT[:, 64 * nb : 64 * (nb + 1), :],
                    in_=ps[:C, :].rearrange("p (t b) -> p t b", b=GB),
                )
        # kick off the em-partial exchange, then do all label-only score work
        # while the collective is in flight
        nc.sync.dma_start(out=ctrbE.ap().rearrange("c (t b) -> c t b", b=GB), in_=emT[:])
        nc.gpsimd.collective_compute(
            "AllGather",
            ALU.bypass,
            replica_groups=PAIRS,
            ins=[ctrbE.ap()],
            outs=[hallE.ap()],
        )
        nc.gpsimd.indirect_dma_start(
            out=xpem[:].rearrange("c t b -> c (t b)"),
            out_offset=None,
            in_=hallE.ap().rearrange("r c n -> (r c) n"),
            in_offset=IndirectOffsetOnAxis(ap=gidxE_sb[:, 0:1], axis=0),
        )

        # ---- CRF ----
        with tc.tile_pool(name="crfw", bufs=1) as cw, tc.tile_pool(
            name="crfps", bufs=1, space="PSUM"
        ) as cps:
            emv = emT[:].rearrange("c t b -> c (t b)")
            xrev3 = xpem[:, ::-1, :]
            Q4 = cw.tile([C, TP, GB], F32, tag="q4", name="Q4")
            nc.vector.memset(Q4[:, 0:CW, :], 1.0)
            nc.vector.memset(Q4[:, CW + T :, :], 1.0)

            # labels broadcast across C partitions (t-major)
            lab_bc = cw.tile([C, NT], F32, tag="labbc", name="lab_bc")
            for nb in range(4):
                bps = cps.tile([C, 512], F32, tag="cps512")
                nc.tensor.matmul(
                    out=bps[:],
                    lhsT=ones1C[:],
                    rhs=labf_sb[:, 512 * nb : 512 * (nb + 1)],
                    start=True,
                    stop=True,
                )
                nc.vector.tensor_copy(out=lab_bc[:, 512 * nb : 512 * (nb + 1)], in_=bps[:])
            OH = cw.tile([C, NT], F32, tag="oh", name="OH")
            nc.vector.tensor_scalar(
                out=OH[:], in0=lab_bc[:], scalar1=iota_sb[:], scalar2=None, op0=ALU.is_equal
            )

            # transition pairs (t-major: pair (t,t+1) = cols n, n+GB)
            M1 = cw.tile([C, NT], F32, tag="m1", name="M1")
            for nb in range(4):
                lo = 512 * nb
                hi = min(512 * (nb + 1), NT - GB)
                mps = cps.tile([C, 512], F32, tag="cps512")
                nc.tensor.matmul(
                    out=mps[:, : hi - lo],
                    lhsT=transT_sb[:],
                    rhs=OH[:, lo + GB : hi + GB],
                    start=True,
                    stop=True,
                )
                nc.vector.tensor_copy(out=M1[:, lo:hi], in_=mps[:, : hi - lo])
            nc.vector.tensor_tensor(
                out=M1[:, : NT - GB], in0=OH[:, : NT - GB], in1=M1[:, : NT - GB], op=ALU.mult
            )
            pd_r = cw.tile([C, 1], F32, tag="pdr")
            nc.vector.reduce_sum(out=pd_r[:], in_=M1[:, : NT - GB], axis=mybir.AxisListType.X)

            # start/end terms
            OHv = OH[:].rearrange("c (t b) -> c t b", b=GB)
            st8 = cw.tile([C, GB], F32, tag="st8")
            nc.vector.tensor_scalar_mul(out=st8[:], in0=OHv[:, 0, :], scalar1=stv_sb[:])
            st_r = cw.tile([C, 1], F32, tag="str")
            nc.vector.reduce_sum(out=st_r[:], in_=st8[:], axis=mybir.AxisListType.X)
            en8 = cw.tile([C, GB], F32, tag="en8")
            nc.vector.tensor_scalar_mul(out=en8[:], in0=OHv[:, T - 1, :], scalar1=env_sb[:])
            en_r = cw.tile([C, 1], F32, tag="enr")
            nc.vector.reduce_sum(out=en_r[:], in_=en8[:], axis=mybir.AxisListType.X)

            # scaled em (em_total + offsets); gold is taken from this tensor
            # and the known offset sum is corrected on the host
            emsc = cw.tile([C, NT], F32, tag="emsc")
            nc.vector.tensor_tensor(out=emsc[:], in0=emv, in1=offq_sb[:], op=ALU.add)
            nc.vector.tensor_tensor(
                out=emsc[:].rearrange("c (t b) -> c t b", b=GB),
                in0=emsc[:].rearrange("c (t b) -> c t b", b=GB),
                in1=xrev3,
                op=ALU.add,
            )
            nc.scalar.activation(
                out=Q4[:, CW : CW + T, :].rearrange("c t b -> c (t b)"),
                in_=emsc[:], func=AF.Exp,
            )
            dve_touch(Q4[0:1, 0, 0:1])
            gem = lab_bc  # reuse
            nc.vector.tensor_tensor(out=gem[:], in0=emsc[:], in1=OH[:], op=ALU.mult)
            gem_r = cw.tile([C, 1], F32, tag="gred")
            nc.vector.reduce_sum(out=gem_r[:], in_=gem[:], axis=mybir.AxisListType.X)

            score_ps = cps.tile([1, 8], F32, tag="scoreps")
            for i, r in enumerate((gem_r, pd_r, st_r, en_r)):
                nc.tensor.matmul(
                    out=score_ps[:1, :1],
                    lhsT=onesC1[:],
                    rhs=r[:],
                    start=(i == 0),
                    stop=(i == 3),
                    skip_group_check=True,
                )
            score_sb = cw.tile([1, 1], F32, tag="scoresb")
            nc.vector.tensor_copy(out=score_sb[:], in_=score_ps[:1, :1])

            # ---- chunked rank-1 alpha/beta chains ----
            # alpha: column block k tracks padded index pa = CCL*k + tau,
            #        i.e. real t = CCL*k + tau - CW.
            # beta:  column block k tracks pa = CCL*k + 2*CW + CCL - 1 - tau.
            NCOL = CK * GB  # 128
            Qp = Q4[:]

            def qslice(base_pa):
                # [C, CK, GB] strided gather: chunk k at padded index base_pa + CCL*k
                return Qp[:, base_pa : base_pa + CCL * (CK - 1) + 1 : CCL, :]

            A = [cw.tile([C, CK, GB], F32, tag=f"av{p}", name=f"A{p}") for p in range(2)]
            Bv = [cw.tile([C, CK, GB], F32, tag=f"bv{p}", name=f"B{p}") for p in range(2)]
            Asnap = cw.tile([C, CK, GB], F32, tag="asnap", name="Asnap")

            # init at tau=0; at tau, alpha chunk k sits at real t = CCL*k + tau - CW
            # and beta chunk k at real t = CCL*k + CW + CCL - 1 - tau.
            nc.vector.tensor_copy(out=A[0][:], in_=qslice(0))
            nc.vector.tensor_copy(out=Bv[0][:], in_=qslice(2 * CW + CCL - 1))
            ac, bc = A[0], Bv[0]
            for tau in range(1, CW + CCL + 1):
                # alpha step: a <- (E^T a) * q[pa = CCL*k + tau]
                aps = cps.tile([C, CK, GB], F32, tag="aps", bufs=2)
                nc.tensor.matmul(
                    out=aps[:].rearrange("c k b -> c (k b)"),
                    lhsT=E_sb[:],
                    rhs=ac[:].rearrange("c k b -> c (k b)"),
                    start=True,
                    stop=True,
                )
                an = A[tau & 1]
                nc.vector.tensor_tensor(out=an[:], in0=aps[:], in1=qslice(tau), op=ALU.mult)
                ac = an
                if tau <= CW + CCL - 1:
                    # beta step: b <- E (q[pa+1] * b)
                    wq = cw.tile([C, CK, GB], F32, tag="wq", bufs=2)
                    nc.vector.tensor_tensor(
                        out=wq[:], in0=bc[:], in1=qslice(2 * CW + CCL - tau), op=ALU.mult
                    )
                    bps = cps.tile([C, CK, GB], F32, tag="bps", bufs=2)
                    nc.tensor.matmul(
                        out=bps[:].rearrange("c k b -> c (k b)"),
                        lhsT=ET_sb[:],
                        rhs=wq[:].rearrange("c k b -> c (k b)"),
                        start=True,
                        stop=True,
                    )
                    bn = Bv[tau & 1]
                    nc.vector.tensor_copy(out=bn[:], in_=bps[:])
                    bc = bn
                if tau == CW:
                    # alpha chunk 0 reaches its true start (t=0): exact reset,
                    # then snapshot a'_k(t_k) for the s_k meets.
                    nc.vector.tensor_scalar_mul(
                        out=ac[:, 0, :], in0=Qp[:, CW, :], scalar1=expst_sb[:]
                    )
                    nc.vector.tensor_copy(out=Asnap[:], in_=ac[:])
                    # beta chunk K-1 reaches t = T-1: reset to exp(end)
                    nc.vector.tensor_copy(out=bc[:, CK - 1, :], in_=expen8_sb[:])

            # r_k = a'ext_{k-1}(t_k) . b'_k(t_k): final ac column k-1 vs final
            # bc column k;  s_k = a'_k(t_k) . b'_k(t_k) from the Asnap columns.
            rmul = cw.tile([C, CK - 1, GB], F32, tag="rmul")
            nc.vector.tensor_tensor(
                out=rmul[:], in0=ac[:, 0 : CK - 1, :], in1=bc[:, 1:CK, :], op=ALU.mult
            )
            smul = cw.tile([C, CK - 2, GB], F32, tag="smul")
            nc.vector.tensor_tensor(
                out=smul[:], in0=Asnap[:, 1 : CK - 1, :], in1=bc[:, 1 : CK - 1, :], op=ALU.mult
            )
            rs_ps = cps.tile([1, (2 * CK - 3) * GB], F32, tag="rsps")
            nc.tensor.matmul(
                out=rs_ps[:, 0 : (CK - 1) * GB],
                lhsT=onesC1[:],
                rhs=rmul[:].rearrange("c k b -> c (k b)"),
                start=True,
                stop=False,
                skip_group_check=True,
            )
            nc.tensor.matmul(
                out=rs_ps[:, (CK - 1) * GB :],
                lhsT=onesC1[:],
                rhs=smul[:].rearrange("c k b -> c (k b)"),
                start=False,
                stop=True,
                skip_group_check=True,
            )
            rs_sb = cw.tile([1, (2 * CK - 3) * GB], F32, tag="rssb")
            nc.vector.tensor_copy(out=rs_sb[:], in_=rs_ps[:])
            lrs = cw.tile([1, (2 * CK - 3) * GB], F32, tag="lrs")
            nc.scalar.activation(out=lrs[:], in_=rs_sb[:], func=AF.Ln)
            lr_tot = cw.tile([1, 1], F32, tag="lrtot")
            nc.vector.reduce_sum(
                out=lr_tot[:], in_=lrs[:, 0 : (CK - 1) * GB], axis=mybir.AxisListType.X
            )
            ls_tot = cw.tile([1, 1], F32, tag="lstot")
            nc.vector.reduce_sum(
                out=ls_tot[:], in_=lrs[:, (CK - 1) * GB :], axis=mybir.AxisListType.X
            )
            lz_tot = cw.tile([1, 1], F32, tag="lztot")
            nc.vector.tensor_tensor(out=lz_tot[:], in0=lr_tot[:], in1=ls_tot[:], op=ALU.subtract)
            loss_sb = cw.tile([1, 2], F32, tag="loss_sb")
            nc.vector.tensor_tensor(
                out=loss_sb[:, 0:1], in0=lz_tot[:], in1=score_sb[:], op=ALU.subtract
            )
            nc.vector.tensor_copy(out=loss_sb[:, 1:2], in_=lz_tot[:])
            nc.sync.dma_start(out=loss_out, in_=loss_sb[:])
        crf_cm.__exit__(None, None, None)

    est.close()


# ---------------------------------------------------------------------------
# host side
# ---------------------------------------------------------------------------

def _crf_static_corr():
    """Total static ln-scale correction for the chunked CRF (per core).

    Chains consume q~ = q * 2^-24 at real tokens t % 6 == 5 (padding = 1, no
    scale). lnZ_dev = sum_k ln r_k - sum_k ln s_k misses corr where
      ln r_k/s_k(true) = ln r_k/s_k(dev) - (consumed scale sums).
    Correction per sequence: lnZ_true = lnZ_dev + corr.
    """
    sig = np.zeros(T)
    sig[np.arange(T) % 6 == 5] = -CRF_SCALE_BITS * LOG2

    def a_sig(k, t_end):
        t0 = 0 if k == 0 else CCL * k - CW
        return sig[max(0, t0) : t_end + 1].sum()

    def b_sig(k):
        tk = CCL * k
        hi = T - 1 if k == CK - 1 else min(T - 1, tk + CCL - 1 + CW)
        return sig[tk + 1 : hi + 1].sum()

    corr = 0.0
    for k in range(1, CK):
        corr += a_sig(k - 1, CCL * k) + b_sig(k)
        if k <= CK - 2:
            corr -= a_sig(k, CCL * k) + b_sig(k)
    # lnZ_true = lnZ_dev - corr  (dev logs include the scale sums)
    return -corr


def make_in_maps(inputs):
    ids = np.asarray(inputs["input_ids"]).astype(np.int64)
    labels = np.asarray(inputs["labels"]).astype(np.int64)
    word_emb = _f32(inputs["word_emb"])
    pos_emb = _f32(inputs["pos_emb"])
    type_emb = _f32(inputs["type_emb"])
    ln_g = _f32(inputs["ln_g"])
    ln_b = _f32(inputs["ln_b"])
    w_ih = _f32(inputs["w_ih"])
    w_hh = _f32(inputs["w_hh"])
    b_ih = _f32(inputs["b_ih"])
    b_hh = _f32(inputs["b_hh"])
    fc_w = _f32(inputs["fc_w"])
    fc_b = _f32(inputs["fc_b"])
    crf_start = _f32(inputs["crf_start"])
    crf_end = _f32(inputs["crf_end"])
    crf_trans = _f32(inputs["crf_trans"])

    posty0 = pos_emb[:T] + type_emb[0][None, :]
    gsl = slice(2 * HD, 3 * HD)  # g-gate rows

    # ln-scale offsets per token (t-major) + fc bias per class (em is sent
    # around bias-free; the bias only matters inside exp(em + off))
    offt = np.zeros(T, np.float32)
    offt[np.arange(T) % 6 == 5] = -CRF_SCALE_BITS * LOG2
    offq = (
        np.repeat(offt, GB)[None, :] + fc_b[:, None]
    ).astype(np.float32)

    in_maps = []
    for core in range(NCORES):
        g, d = core // 2, core % 2
        sl = slice(GB * g, GB * (g + 1))
        ids_loc = ids[sl]
        lab_loc = labels[sl]
        posty = posty0
        if d == 1:
            ids_loc = ids_loc[:, ::-1]
            lab_loc = lab_loc[:, ::-1]
            posty = posty0[::-1]

        # layer-0 weights with LN affine folded in
        w0 = w_ih[0, d] * ln_g[None, :]
        bias0 = b_ih[0, d] + b_hh[0, d] + w_ih[0, d] @ ln_b
        # layer-1 weights, columns permuted to local [own, partner] order
        w1 = w_ih[1, d]
        if d == 1:
            w1 = np.concatenate([w1[:, HD:], w1[:, :HD]], axis=1)
        bias1 = b_ih[1, d] + b_hh[1, d]
        wh0 = w_hh[0, d].copy()
        wh1 = w_hh[1, d].copy()
        # scale g-gate rows x2 (tanh(g) = 2*sigma(2g) - 1 on device)
        w0 = w0.copy()
        w1 = w1.copy()
        bias0 = bias0.copy()
        bias1 = bias1.copy()
        w0[gsl] *= 2.0
        w1[gsl] *= 2.0
        wh0[gsl] *= 2.0
        wh1[gsl] *= 2.0
        bias0[gsl] *= 2.0
        bias1[gsl] *= 2.0

        fcp = fc_w if d == 0 else np.concatenate([fc_w[:, HD:], fc_w[:, :HD]], axis=1)

        trans_eff = crf_trans if d == 0 else crf_trans.T
        start_eff = crf_start if d == 0 else crf_end
        end_eff = crf_end if d == 0 else crf_start

        pr = 1 - d
        gidx = np.empty((128, KCH), np.int32)
        for cch in range(KCH):
            gidx[:, cch] = pr * (KCH * 128) + cch * 128 + np.arange(128)
        gidxE = (pr * C + np.arange(C, dtype=np.int32)).reshape(C, 1)

        cpack = np.zeros((C, 64), np.float32)
        cpack[:, 0:C] = np.exp(trans_eff)
        cpack[:, C : 2 * C] = np.exp(trans_eff).T
        cpack[:, 2 * C : 3 * C] = trans_eff.T
        cpack[:, 42] = np.exp(start_eff)
        cpack[:, 43] = start_eff
        cpack[:, 44] = end_eff
        cpack[:, 45] = np.arange(C, dtype=np.float32)
        cpack[:, 48:56] = np.exp(end_eff)[:, None]

        b01 = np.concatenate(
            [bias0.reshape(MCH, 128).T, bias1.reshape(MCH, 128).T], axis=1
        )

        labf = np.ascontiguousarray(lab_loc.T.reshape(1, NT).astype(np.float32))

        in_maps.append(
            dict(
                ids32=np.ascontiguousarray(ids_loc.reshape(NT, 1).astype(np.int32)),
                labf=labf,
                word_emb=word_emb,
                posty=np.ascontiguousarray(posty),
                wih0T=np.ascontiguousarray(w0.T.astype(np.float32)).astype(
                    ml_dtypes.float8_e4m3
                ),
                wih1oT=_bf(w1.T[:HD]),
                wih1pT=np.ascontiguousarray(w1.T[HD:].astype(np.float32)).astype(
                    ml_dtypes.float8_e4m3
                ),
                whh0T=_bf(wh0.T),
                whh1T=_bf(wh1.T),
                b01=np.ascontiguousarray(b01),
                fcT=_bf(fcp.T),
                cpack=cpack,
                offq=np.ascontiguousarray(offq),
                gidx=gidx,
                gidxE=gidxE,
            )
        )
    return in_maps


_PROGRAM = None
_COST_MODEL_NS = None


def _get_program():
    global _PROGRAM, _COST_MODEL_NS
    if _PROGRAM is None:
        _PROGRAM = build_program()
        try:
            from concourse.timeline_sim import TimelineSim

            _COST_MODEL_NS = int(TimelineSim(_PROGRAM, trace=False, no_exec=True).simulate())
        except Exception:
            _COST_MODEL_NS = None
    return _PROGRAM


def run(inputs, trace=False):
    nc = _get_program()
    in_maps = make_in_maps(inputs)
    res = run_bass_kernel_spmd(nc, in_maps, core_ids=list(range(NCORES)), trace=trace)
    # per-core static corrections: chunked-CRF scale bookkeeping, plus the
    # ln-scale offsets that ride into the gold score via emsc = em + offq
    offt_sum = -CRF_SCALE_BITS * LOG2 * float(np.sum(np.arange(T) % 6 == 5))
    corr = (_crf_static_corr() + offt_sum) * GB
    total = np.float64(0.0)
    for g in range(4):
        total += np.float64(res.results[2 * g]["loss"][0, 0]) + np.float64(corr)
    return np.asarray(total, dtype=np.float32), res


def kernel(**inputs):
    out, _ = run(inputs, trace=False)
    return out
